# revision 14
# baseline (speedup 1.0000x reference)
"""ContrastiveAttentionCompensation on 8 TRN2 NeuronCores (Bass/Tile).

Reference computation (N = M = 8192, D = 512, fp32):
    q = h1 @ Wq.T + bq                  [N, D]
    k = h2 @ Wk.T + bk                  [M, D]
    attn = (q @ k.T) / sqrt(D)          [N, M]
    soft_text = softmax(attn, axis=-1)  row softmax
    soft_img  = softmax(attn, axis=0)   column softmax
    fused1 = soft_text @ k + q          [N, D]
    fused2 = soft_img.T @ q + k         [M, D]
    returns (fused1, fused2, attn)

Sharding: rows of h1 (N dim of the score matrix) across 8 cores. Each core
computes its [1024, 8192] slab of attn / E = exp(attn):
  - row softmax is core-local (full M per core); fused1 = (E@k)/row_sum + q
    is core-local.
  - fused2 needs sum over N of E[i,j] q[i,d] -> per-core partials P2[j,d] and
    column sums, reduced with chunked ReduceScatters (colsum rides as column
    512 of the 520-wide RS buffer).
  - k projection computed sharded; kT and k-natural are AllGathered (f32r).
Softmax skips max subtraction (attn is O(6); exp is safe in fp32 and matches
jax.nn.softmax to fp32 accuracy).

Matmul dtype: float32r (1 cyc/row) by default; float32 would be 4 cyc/row.
"""
import sys

sys.path.insert(0, "/opt/trn_rl_repo")

import numpy as np

N, M, D = 8192, 8192, 512
NCORES = 8
NLOC = N // NCORES          # 1024 rows per core
P = 128
NIT = NLOC // P             # 8 i-tiles per core
PANEL = 1024                # j-panel width
NPANELS = M // PANEL        # 8
NJT = PANEL // P            # 8 j-tiles per panel
CHUNK_PANELS = (3, 3, 1, 1)  # ReduceScatter chunking (panels per chunk)
NCHUNKS = len(CHUNK_PANELS)
CHUNK_BASE = [sum(CHUNK_PANELS[:k]) * PANEL for k in range(NCHUNKS + 1)]
WPAD = 520                  # 512 d-cols + colsum col (512) + pad to 32B

_nc_cache = {}


def _build_nc():
    import concourse.bass as bass
    import concourse.mybir as mybir
    import concourse.tile as tile
    from concourse import bacc
    from concourse.masks import make_identity

    F32 = mybir.dt.float32
    F32R = mybir.dt.float32r
    MM_DT = F32R
    AF = mybir.ActivationFunctionType
    ALU = mybir.AluOpType

    nc = bacc.Bacc(None, num_devices=NCORES)

    h1 = nc.declare_dram_parameter("h1", [NLOC, D], F32, isOutput=False)
    h2 = nc.declare_dram_parameter("h2", [NLOC, D], F32, isOutput=False)
    wqt_s = nc.declare_dram_parameter("wqt_s", [D, D], F32, isOutput=False)
    wqt = nc.declare_dram_parameter("wqt", [D, D], F32, isOutput=False)
    wkt = nc.declare_dram_parameter("wkt", [D, D], F32, isOutput=False)
    bq_s = nc.declare_dram_parameter("bq_s", [1, D], F32, isOutput=False)
    bq = nc.declare_dram_parameter("bq", [1, D], F32, isOutput=False)
    bk = nc.declare_dram_parameter("bk", [1, D], F32, isOutput=False)

    attn_o = nc.declare_dram_parameter("attn", [NLOC, M], F32, isOutput=True)
    f1_o = nc.declare_dram_parameter("f1", [NLOC, D], F32, isOutput=True)
    f2_o = nc.declare_dram_parameter("f2", [NLOC, D], F32, isOutput=True)

    def bcast_row(ap_1d, parts=P):
        return bass.AP(tensor=ap_1d.tensor, offset=ap_1d.offset,
                       ap=[[0, parts]] + ap_1d.ap[1:])

    with tile.TileContext(nc) as tc:
        with (
            tc.tile_pool(name="persist", bufs=1) as pers,
            tc.tile_pool(name="dram", bufs=1, space="DRAM") as dram,
        ):
            idr = pers.tile([P, P], MM_DT)
            idf = pers.tile([P, P], F32)
            make_identity(nc, idf)
            nc.vector.tensor_copy(idr, idf)

            qt_s = pers.tile([P, 4, NLOC], MM_DT)     # qT' [d, i] scaled+bias
            q_sb = pers.tile([P, NIT, D], MM_DT)      # q natural [i, d]
            o1_acc = pers.tile([P, NIT, D], F32)      # E @ k accumulator
            rowsum = pers.tile([P, NIT, NPANELS], F32)
            colsum = pers.tile([P, NJT, NPANELS], F32)
            bqs_pt = pers.tile([P, 4], F32)
            bk_pt = pers.tile([P, 4], F32)
            bq_bc = pers.tile([P, D], F32)
            bk_bc = pers.tile([P, D], F32)

            nc.sync.dma_start(out=bqs_pt, in_=bq_s[0, :].rearrange("(t p) -> p t", p=P))
            nc.sync.dma_start(out=bk_pt, in_=bk[0, :].rearrange("(t p) -> p t", p=P))
            nc.sync.dma_start(out=bq_bc, in_=bcast_row(bq[0:1, :]))
            nc.sync.dma_start(out=bk_bc, in_=bcast_row(bk[0:1, :]))

            ag_in = dram.tile([D, NLOC], MM_DT)                       # kT shard
            ag_out = dram.tile([NCORES, D, NLOC], MM_DT, addr_space="Shared")
            ag2_in = dram.tile([NLOC, D], MM_DT)                      # k shard
            ag2_out = dram.tile([NCORES, NLOC, D], MM_DT, addr_space="Shared")
            p2b = [dram.tile([CHUNK_PANELS[k] * PANEL, WPAD], F32, name=f"p2b{k}")
                   for k in range(NCHUNKS)]
            rs_out = [dram.tile([CHUNK_PANELS[k] * PANEL // NCORES, WPAD], F32,
                                name=f"rso{k}") for k in range(NCHUNKS)]

            # ================= precompute =================
            with (
                tc.tile_pool(name="pre", bufs=1) as pre,
                tc.tile_pool(name="preps", bufs=4, space="PSUM") as preps,
            ):
                # --- h2 side first: get the AllGathers in flight ASAP
                h2_sb = pre.tile([P, NIT, D], F32)
                w3 = pre.tile([P, 4, D], MM_DT, name="w3")
                w3f = pre.tile([P, 4, D], F32, name="w3f")
                nc.sync.dma_start(out=h2_sb, in_=h2[:, :].rearrange("(t p) d -> p t d", p=P))
                nc.sync.dma_start(out=w3f, in_=wkt[:, :].rearrange("(t p) d -> p t d", p=P))
                nc.vector.tensor_copy(w3, w3f)

                h2t = pre.tile([P, 4, NLOC], MM_DT)
                for it in range(NIT):
                    tp2 = preps.tile([P, D], F32, name="tp2", tag="pre")
                    for ct in range(4):
                        nc.tensor.transpose(tp2[:, ct * P:(ct + 1) * P],
                                            h2_sb[:, it, ct * P:(ct + 1) * P], idf)
                    nc.vector.tensor_copy(
                        h2t.rearrange("p c (t f) -> p c t f", f=P)[:, :, it, :],
                        tp2.rearrange("p (c f) -> p c f", f=P))

                # kT shard [d, j_loc] -> AG1
                kts = pre.tile([P, 4, NLOC], MM_DT)
                for dt in range(4):
                    for jh in range(2):
                        pk = preps.tile([P, 512], F32, name="pk", tag="pre")
                        for ct in range(4):
                            nc.tensor.matmul(
                                pk, w3[:, ct, dt * P:(dt + 1) * P],
                                h2t[:, ct, jh * 512:(jh + 1) * 512],
                                start=(ct == 0), stop=(ct == 3))
                        nc.scalar.activation(
                            out=kts[:, dt, jh * 512:(jh + 1) * 512], in_=pk,
                            func=AF.Identity, bias=bk_pt[:, dt:dt + 1])
                nc.sync.dma_start(
                    out=ag_in[:, :].rearrange("(t p) j -> p t j", p=P), in_=kts)
                nc.gpsimd.collective_compute(
                    "AllGather", mybir.AluOpType.bypass,
                    replica_groups=[list(range(NCORES))],
                    ins=[ag_in[:].opt()], outs=[ag_out[:].opt()])

                # k natural shard [j_loc, d] -> AG2
                knat = pre.tile([P, NIT, D], MM_DT)
                for jt in range(NIT):
                    pk2 = preps.tile([P, D], F32, name="pk2", tag="pre")
                    for ct in range(4):
                        nc.tensor.matmul(pk2, h2t[:, ct, jt * P:(jt + 1) * P],
                                         w3[:, ct, :], start=(ct == 0), stop=(ct == 3))
                    nc.vector.tensor_add(knat[:, jt, :], pk2, bk_bc)
                nc.sync.dma_start(
                    out=ag2_in[:, :].rearrange("(t p) d -> p t d", p=P), in_=knat)
                nc.gpsimd.collective_compute(
                    "AllGather", mybir.AluOpType.bypass,
                    replica_groups=[list(range(NCORES))],
                    ins=[ag2_in[:].opt()], outs=[ag2_out[:].opt()])

                # --- h1 side (overlaps the AllGathers)
                h1_sb = pre.tile([P, NIT, D], F32)
                w1 = pre.tile([P, 4, D], MM_DT, name="w1")
                w2 = pre.tile([P, 4, D], MM_DT, name="w2")
                w1f = pre.tile([P, 4, D], F32, name="w1f")
                w2f = pre.tile([P, 4, D], F32, name="w2f")
                nc.sync.dma_start(out=h1_sb, in_=h1[:, :].rearrange("(t p) d -> p t d", p=P))
                for wf, wr, src in ((w1f, w1, wqt_s), (w2f, w2, wqt)):
                    nc.sync.dma_start(out=wf, in_=src[:, :].rearrange("(t p) d -> p t d", p=P))
                    nc.vector.tensor_copy(wr, wf)

                h1t = pre.tile([P, 4, NLOC], MM_DT)
                for it in range(NIT):
                    tp1 = preps.tile([P, D], F32, name="tp1", tag="pre")
                    for ct in range(4):
                        nc.tensor.transpose(tp1[:, ct * P:(ct + 1) * P],
                                            h1_sb[:, it, ct * P:(ct + 1) * P], idf)
                    nc.vector.tensor_copy(
                        h1t.rearrange("p c (t f) -> p c t f", f=P)[:, :, it, :],
                        tp1.rearrange("p (c f) -> p c f", f=P))

                for dt in range(4):
                    for ih in range(2):
                        pp = preps.tile([P, 512], F32, name="pp", tag="pre")
                        for ct in range(4):
                            nc.tensor.matmul(
                                pp, w1[:, ct, dt * P:(dt + 1) * P],
                                h1t[:, ct, ih * 512:(ih + 1) * 512],
                                start=(ct == 0), stop=(ct == 3))
                        nc.scalar.activation(
                            out=qt_s[:, dt, ih * 512:(ih + 1) * 512], in_=pp,
                            func=AF.Identity, bias=bqs_pt[:, dt:dt + 1])

                for it in range(NIT):
                    pq = preps.tile([P, D], F32, name="pq", tag="pre")
                    for ct in range(4):
                        nc.tensor.matmul(pq, h1t[:, ct, it * P:(it + 1) * P],
                                         w2[:, ct, :], start=(ct == 0), stop=(ct == 3))
                    nc.vector.tensor_add(q_sb[:, it, :], pq, bq_bc)

            rid = nc.gpsimd.partition_id()

            o1_gate = {}
            # ================= main j-panel loop =================
            with (
                tc.tile_pool(name="work", bufs=1) as work,
                tc.tile_pool(name="psA", bufs=2, space="PSUM") as psA,   # attn (2x2 banks)
                tc.tile_pool(name="psT", bufs=1, space="PSUM") as psT,   # ET transp (2 banks)
                tc.tile_pool(name="psM", bufs=2, space="PSUM") as psM,   # mm2/mm3 (2 banks)
            ):
                for p in range(NPANELS):
                    j0 = p * PANEL
                    ch = next(k for k in range(NCHUNKS)
                              if CHUNK_BASE[k] <= j0 < CHUNK_BASE[k + 1])
                    ktp_t = []
                    for dt in range(4):
                        ktp = work.tile([P, PANEL], MM_DT, name="ktp", tag="ktp", bufs=6)
                        nc.sync.dma_start(out=ktp, in_=ag_out[p, dt * P:(dt + 1) * P, :])
                        ktp_t.append(ktp)
                    ksb_t = []
                    for jt in range(NJT):
                        ksb = work.tile([P, D], MM_DT, name="ksb", tag="ksb", bufs=10)
                        nc.sync.dma_start(out=ksb, in_=ag2_out[p, jt * P:(jt + 1) * P, :])
                        ksb_t.append(ksb)

                    # attn matmuls + exp + attn output
                    e_t = []
                    for it in range(NIT):
                        pa = psA.tile([P, PANEL], F32, name="pa", tag="pa")
                        for dt in range(4):
                            for jh in range(2):
                                nc.tensor.matmul(
                                    pa[:, jh * 512:(jh + 1) * 512],
                                    qt_s[:, dt, it * P:(it + 1) * P],
                                    ktp_t[dt][:, jh * 512:(jh + 1) * 512],
                                    start=(dt == 0), stop=(dt == 3))
                        e = work.tile([P, PANEL], MM_DT, name="e", tag="e", bufs=9)
                        nc.scalar.activation(out=e, in_=pa, func=AF.Exp,
                                             accum_out=rowsum[:, it, p:p + 1])
                        e_t.append(e)
                        ao = work.tile([P, PANEL], F32, name="ao", tag="ao", bufs=3)
                        nc.vector.tensor_copy(ao, pa)
                        nc.sync.dma_start(
                            out=attn_o[it * P:(it + 1) * P, j0:j0 + PANEL], in_=ao)

                    # ET = E.T (PE transposes); colsum via ACT copy accum
                    et_t = []
                    for jt in range(NJT):
                        pt = psT.tile([P, NLOC], MM_DT, name="pt", tag="pt")
                        for it in range(NIT):
                            nc.tensor.transpose(pt[:, it * P:(it + 1) * P],
                                                e_t[it][:, jt * P:(jt + 1) * P], idr)
                        et = work.tile([P, NLOC], MM_DT, name="et", tag="et", bufs=9)
                        nc.scalar.activation(out=et, in_=pt, func=AF.Identity,
                                             accum_out=colsum[:, jt, p:p + 1])
                        et_t.append(et)

                    # mm2: P2[j, d] partials -> p2 bounce
                    for jt in range(NJT):
                        pm = psM.tile([P, D], F32, name="pm", tag="pm")
                        for it in range(NIT):
                            nc.tensor.matmul(pm, e_t[it][:, jt * P:(jt + 1) * P],
                                             q_sb[:, it, :], start=(it == 0),
                                             stop=(it == NIT - 1))
                        p2s = work.tile([P, D], F32, name="p2s", tag="p2s", bufs=2)
                        nc.vector.tensor_copy(p2s, pm)
                        r0 = j0 - CHUNK_BASE[ch] + jt * P
                        nc.sync.dma_start(out=p2b[ch][r0:r0 + P, 0:D], in_=p2s)

                    # mm3: out1[i, d] += E @ k (lhsT = ET)
                    for it in range(NIT):
                        pm = psM.tile([P, D], F32, name="pm3", tag="pm")
                        for jt in range(NJT):
                            nc.tensor.matmul(pm, et_t[jt][:, it * P:(it + 1) * P],
                                             ksb_t[jt], start=(jt == 0),
                                             stop=(jt == NJT - 1))
                        if p == 0:
                            o1_gate[p] = nc.vector.tensor_copy(o1_acc[:, it, :], pm)
                        else:
                            o1_gate[p] = nc.vector.tensor_add(
                                o1_acc[:, it, :], pm, o1_acc[:, it, :])

                    # colsum column for this panel
                    dst = bass.AP(
                        tensor=p2b[ch].tensor,
                        offset=p2b[ch].offset + (j0 - CHUNK_BASE[ch]) * WPAD + D,
                        ap=[[WPAD, P], [P * WPAD, NJT]])
                    nc.sync.dma_start(out=dst, in_=colsum[:, :, p])

                    if j0 + PANEL == CHUNK_BASE[ch + 1]:
                        nc.gpsimd.collective_compute(
                            "ReduceScatter", mybir.AluOpType.add,
                            replica_groups=[list(range(NCORES))],
                            ins=[p2b[ch][:].opt()], outs=[rs_out[ch][:].opt()])

                # ================= finalize =================
                # fused1 (DVE, core-local)
                rs_tot = work.tile([P, NIT], F32)
                nc.vector.tensor_reduce(rs_tot, rowsum, axis=mybir.AxisListType.X,
                                        op=mybir.AluOpType.add)
                rs_rec = work.tile([P, NIT], F32)
                nc.vector.reciprocal(rs_rec, rs_tot)
                for it in range(NIT):
                    f1s = work.tile([P, D], F32, name="f1s", tag="f1s", bufs=2)
                    nc.vector.tensor_scalar(
                        out=f1s, in0=o1_acc[:, it, :], scalar1=rs_rec[:, it:it + 1],
                        scalar2=None, op0=mybir.AluOpType.mult)
                    nc.vector.tensor_add(f1s, f1s, q_sb[:, it, :].bitcast(F32))
                    nc.sync.dma_start(out=f1_o[it * P:(it + 1) * P, :], in_=f1s)

                # fused2 per chunk (gpsimd only: don't stall DVE/PE on the RS)
                ag2_flat = ag2_out.rearrange("c j d -> (c j) d")
                out_r0 = 0
                from concourse.tile_rust import add_dep_helper

                fin_gate = {0: 5, 1: 7, 2: 7, 3: 7}
                for ck in range(NCHUNKS):
                    nb = CHUNK_PANELS[ck] * PANEL // NCORES // P  # band i-tiles
                    for t in range(nb):
                        rsb = work.tile([P, WPAD], F32, name="rsb", tag="rsb", bufs=3)
                        nc.gpsimd.dma_start(
                            out=rsb,
                            in_=rs_out[ck][t * P:(t + 1) * P, :])
                        km = work.tile([P, D], MM_DT, name="km", tag="km", bufs=3)
                        for r in range(NCORES):
                            g0 = CHUNK_BASE[ck] + (r * nb + t) * P
                            nc.gpsimd.dma_start(
                                out=km, in_=ag2_flat[g0:g0 + P, :], cond=(rid == r))
                        crec = work.tile([P, 1], F32, name="crec", tag="crec", bufs=3)
                        h = nc.vector.reciprocal(crec, rsb[:, D:D + 1])
                        add_dep_helper(h.ins, o1_gate[fin_gate[ck]].ins, False,
                                       "fused2 finalize after late panel")
                        f2s = work.tile([P, D], F32, name="f2s", tag="f2s", bufs=3)
                        nc.vector.tensor_scalar(
                            out=f2s, in0=rsb[:, 0:D], scalar1=crec,
                            scalar2=None, op0=ALU.mult)
                        nc.vector.tensor_add(f2s, f2s, km.bitcast(F32))
                        nc.sync.dma_start(
                            out=f2_o[out_r0:out_r0 + P, :], in_=f2s)
                        out_r0 += P

    nc.compile()
    return nc


def _get_nc():
    if "nc" not in _nc_cache:
        _nc_cache["nc"] = _build_nc()
    return _nc_cache["nc"]


def _make_in_maps(h1, h2, Wq, bq, Wk, bk):
    h1 = np.ascontiguousarray(h1, np.float32)
    h2 = np.ascontiguousarray(h2, np.float32)
    s = np.float32(1.0 / np.sqrt(D))
    wqt = np.ascontiguousarray(np.asarray(Wq, np.float32).T)
    in_common = {
        "wqt_s": wqt * s,
        "wqt": wqt,
        "wkt": np.ascontiguousarray(np.asarray(Wk, np.float32).T),
        "bq_s": (np.asarray(bq, np.float32) * s).reshape(1, D),
        "bq": np.asarray(bq, np.float32).reshape(1, D),
        "bk": np.asarray(bk, np.float32).reshape(1, D),
    }
    return [
        {"h1": h1[c * NLOC:(c + 1) * NLOC], "h2": h2[c * NLOC:(c + 1) * NLOC],
         **in_common}
        for c in range(NCORES)
    ]


def _assemble(res):
    attn = np.concatenate([r["attn"] for r in res], axis=0)
    fused1 = np.concatenate([r["f1"] for r in res], axis=0)
    fused2 = np.empty((M, D), np.float32)
    for c in range(NCORES):
        f2c = res[c]["f2"]
        o = 0
        for ck in range(NCHUNKS):
            nb = CHUNK_PANELS[ck] * PANEL // NCORES
            g0 = CHUNK_BASE[ck] + c * nb
            fused2[g0:g0 + nb] = f2c[o:o + nb]
            o += nb
    return fused1, fused2, attn


def kernel(h1, h2, Wq, bq, Wk, bk):
    from concourse.bass_utils import run_bass_kernel_spmd

    in_maps = _make_in_maps(h1, h2, Wq, bq, Wk, bk)
    nc = _get_nc()
    res = run_bass_kernel_spmd(nc, in_maps, core_ids=list(range(NCORES))).results
    return _assemble(res)


# revision 15
# speedup vs baseline: 1.1429x; 1.1429x over previous
"""ContrastiveAttentionCompensation on 8 TRN2 NeuronCores (Bass/Tile).

Reference computation (N = M = 8192, D = 512, fp32):
    q = h1 @ Wq.T + bq                  [N, D]
    k = h2 @ Wk.T + bk                  [M, D]
    attn = (q @ k.T) / sqrt(D)          [N, M]
    soft_text = softmax(attn, axis=-1)  row softmax
    soft_img  = softmax(attn, axis=0)   column softmax
    fused1 = soft_text @ k + q          [N, D]
    fused2 = soft_img.T @ q + k         [M, D]
    returns (fused1, fused2, attn)

Sharding: rows of h1 (N dim of the score matrix) across 8 cores. Each core
computes its [1024, 8192] slab of attn / E = exp(attn):
  - row softmax is core-local (full M per core); fused1 = (E@k)/row_sum + q
    is core-local.
  - fused2 needs sum over N of E[i,j] q[i,d] -> per-core partials P2[j,d] and
    column sums, reduced with chunked ReduceScatters (colsum rides as column
    512 of the 520-wide RS buffer).
  - k projection computed sharded; kT and k-natural are AllGathered (f32r).
Softmax skips max subtraction (attn is O(6); exp is safe in fp32 and matches
jax.nn.softmax to fp32 accuracy).

Matmul dtype: float32r (1 cyc/row) by default; float32 would be 4 cyc/row.
"""
import sys

sys.path.insert(0, "/opt/trn_rl_repo")

import numpy as np

N, M, D = 8192, 8192, 512
NCORES = 8
NLOC = N // NCORES          # 1024 rows per core
P = 128
NIT = NLOC // P             # 8 i-tiles per core
PANEL = 1024                # j-panel width
NPANELS = M // PANEL        # 8
NJT = PANEL // P            # 8 j-tiles per panel
CHUNK_PANELS = (3, 3, 1, 1)  # ReduceScatter chunking (panels per chunk)
NCHUNKS = len(CHUNK_PANELS)
CHUNK_BASE = [sum(CHUNK_PANELS[:k]) * PANEL for k in range(NCHUNKS + 1)]
WPAD = 520                  # 512 d-cols + colsum col (512) + pad to 32B

_nc_cache = {}


def _build_nc():
    import concourse.bass as bass
    import concourse.mybir as mybir
    import concourse.tile as tile
    from concourse import bacc
    from concourse.masks import make_identity

    F32 = mybir.dt.float32
    F32R = mybir.dt.float32r
    MM_DT = F32R
    AF = mybir.ActivationFunctionType
    ALU = mybir.AluOpType

    nc = bacc.Bacc(None, num_devices=NCORES)

    h1 = nc.declare_dram_parameter("h1", [NLOC, D], F32, isOutput=False)
    h2 = nc.declare_dram_parameter("h2", [NLOC, D], F32, isOutput=False)
    wqt_s = nc.declare_dram_parameter("wqt_s", [D, D], F32, isOutput=False)
    wqt = nc.declare_dram_parameter("wqt", [D, D], F32, isOutput=False)
    wkt = nc.declare_dram_parameter("wkt", [D, D], F32, isOutput=False)
    bq_s = nc.declare_dram_parameter("bq_s", [1, D], F32, isOutput=False)
    bq = nc.declare_dram_parameter("bq", [1, D], F32, isOutput=False)
    bk = nc.declare_dram_parameter("bk", [1, D], F32, isOutput=False)

    attn_o = nc.declare_dram_parameter("attn", [NLOC, M], F32, isOutput=True)
    f1_o = nc.declare_dram_parameter("f1", [NLOC, D], F32, isOutput=True)
    f2_o = nc.declare_dram_parameter("f2", [NLOC, D], F32, isOutput=True)

    def bcast_row(ap_1d, parts=P):
        return bass.AP(tensor=ap_1d.tensor, offset=ap_1d.offset,
                       ap=[[0, parts]] + ap_1d.ap[1:])

    with tile.TileContext(nc) as tc:
        with (
            tc.tile_pool(name="persist", bufs=1) as pers,
            tc.tile_pool(name="dram", bufs=1, space="DRAM") as dram,
        ):
            idr = pers.tile([P, P], MM_DT)
            idf = pers.tile([P, P], F32)
            make_identity(nc, idf)
            nc.vector.tensor_copy(idr, idf)

            qt_s = pers.tile([P, 4, NLOC], MM_DT)     # qT' [d, i] scaled+bias
            q_sb = pers.tile([P, NIT, D], MM_DT)      # q natural [i, d]
            o1_acc = pers.tile([P, NIT, D], F32)      # E @ k accumulator
            rowsum = pers.tile([P, NIT, NPANELS], F32)
            colsum = pers.tile([P, NJT, NPANELS], F32)
            bqs_pt = pers.tile([P, 4], F32)
            bk_pt = pers.tile([P, 4], F32)
            bq_bc = pers.tile([P, D], F32)
            bk_bc = pers.tile([P, D], F32)

            nc.sync.dma_start(out=bqs_pt, in_=bq_s[0, :].rearrange("(t p) -> p t", p=P))
            nc.sync.dma_start(out=bk_pt, in_=bk[0, :].rearrange("(t p) -> p t", p=P))
            nc.sync.dma_start(out=bq_bc, in_=bcast_row(bq[0:1, :]))
            nc.sync.dma_start(out=bk_bc, in_=bcast_row(bk[0:1, :]))

            ag_in = dram.tile([D, NLOC], MM_DT)                       # kT shard
            ag_out = dram.tile([NCORES, D, NLOC], MM_DT, addr_space="Shared")
            ag2_in = dram.tile([NLOC, D], MM_DT)                      # k shard
            ag2_out = dram.tile([NCORES, NLOC, D], MM_DT, addr_space="Shared")
            p2b = [dram.tile([CHUNK_PANELS[k] * PANEL, WPAD], F32, name=f"p2b{k}")
                   for k in range(NCHUNKS)]
            rs_out = [dram.tile([CHUNK_PANELS[k] * PANEL // NCORES, WPAD], F32,
                                name=f"rso{k}") for k in range(NCHUNKS)]

            # ================= precompute =================
            with (
                tc.tile_pool(name="pre", bufs=1) as pre,
                tc.tile_pool(name="preps", bufs=4, space="PSUM") as preps,
            ):
                # --- h2 side first: get the AllGathers in flight ASAP
                h2_sb = pre.tile([P, NIT, D], F32)
                w3 = pre.tile([P, 4, D], MM_DT, name="w3")
                w3f = pre.tile([P, 4, D], F32, name="w3f")
                h2r = h2[:, :].rearrange("(t p) d -> p t d", p=P)
                for it in range(NIT):
                    nc.sync.dma_start(out=h2_sb[:, it, :], in_=h2r[:, it, :])
                w3r = wkt[:, :].rearrange("(t p) d -> p t d", p=P)
                for ct in range(4):
                    nc.sync.dma_start(out=w3f[:, ct, :], in_=w3r[:, ct, :])
                nc.vector.tensor_copy(w3, w3f)

                h2t = pre.tile([P, 4, NLOC], MM_DT)
                for it in range(NIT):
                    tp2 = preps.tile([P, D], F32, name="tp2", tag="pre")
                    for ct in range(4):
                        nc.tensor.transpose(tp2[:, ct * P:(ct + 1) * P],
                                            h2_sb[:, it, ct * P:(ct + 1) * P], idf)
                    nc.vector.tensor_copy(
                        h2t.rearrange("p c (t f) -> p c t f", f=P)[:, :, it, :],
                        tp2.rearrange("p (c f) -> p c f", f=P))

                # kT shard [d, j_loc] -> AG1
                kts = pre.tile([P, 4, NLOC], MM_DT)
                for dt in range(4):
                    for jh in range(2):
                        pk = preps.tile([P, 512], F32, name="pk", tag="pre")
                        for ct in range(4):
                            nc.tensor.matmul(
                                pk, w3[:, ct, dt * P:(dt + 1) * P],
                                h2t[:, ct, jh * 512:(jh + 1) * 512],
                                start=(ct == 0), stop=(ct == 3))
                        nc.scalar.activation(
                            out=kts[:, dt, jh * 512:(jh + 1) * 512], in_=pk,
                            func=AF.Identity, bias=bk_pt[:, dt:dt + 1])
                ag_in_r = ag_in[:, :].rearrange("(t p) j -> p t j", p=P)
                for dt in range(4):
                    for jh in range(2):
                        nc.sync.dma_start(out=ag_in_r[:, dt, jh * 512:(jh + 1) * 512],
                                          in_=kts[:, dt, jh * 512:(jh + 1) * 512])
                nc.gpsimd.collective_compute(
                    "AllGather", mybir.AluOpType.bypass,
                    replica_groups=[list(range(NCORES))],
                    ins=[ag_in[:].opt()], outs=[ag_out[:].opt()])

                # k natural shard [j_loc, d] -> AG2
                knat = pre.tile([P, NIT, D], MM_DT)
                for jt in range(NIT):
                    pk2 = preps.tile([P, D], F32, name="pk2", tag="pre")
                    for ct in range(4):
                        nc.tensor.matmul(pk2, h2t[:, ct, jt * P:(jt + 1) * P],
                                         w3[:, ct, :], start=(ct == 0), stop=(ct == 3))
                    nc.vector.tensor_add(knat[:, jt, :], pk2, bk_bc)
                ag2_in_r = ag2_in[:, :].rearrange("(t p) d -> p t d", p=P)
                for jt in range(NIT):
                    nc.sync.dma_start(out=ag2_in_r[:, jt, :], in_=knat[:, jt, :])
                nc.gpsimd.collective_compute(
                    "AllGather", mybir.AluOpType.bypass,
                    replica_groups=[list(range(NCORES))],
                    ins=[ag2_in[:].opt()], outs=[ag2_out[:].opt()])

                # --- h1 side (overlaps the AllGathers)
                h1_sb = pre.tile([P, NIT, D], F32)
                w1 = pre.tile([P, 4, D], MM_DT, name="w1")
                w2 = pre.tile([P, 4, D], MM_DT, name="w2")
                w1f = pre.tile([P, 4, D], F32, name="w1f")
                w2f = pre.tile([P, 4, D], F32, name="w2f")
                h1r = h1[:, :].rearrange("(t p) d -> p t d", p=P)
                for it in range(NIT):
                    nc.sync.dma_start(out=h1_sb[:, it, :], in_=h1r[:, it, :])
                for wf, wr, src in ((w1f, w1, wqt_s), (w2f, w2, wqt)):
                    wsr = src[:, :].rearrange("(t p) d -> p t d", p=P)
                    for ct in range(4):
                        nc.sync.dma_start(out=wf[:, ct, :], in_=wsr[:, ct, :])
                    nc.vector.tensor_copy(wr, wf)

                h1t = pre.tile([P, 4, NLOC], MM_DT)
                for it in range(NIT):
                    tp1 = preps.tile([P, D], F32, name="tp1", tag="pre")
                    for ct in range(4):
                        nc.tensor.transpose(tp1[:, ct * P:(ct + 1) * P],
                                            h1_sb[:, it, ct * P:(ct + 1) * P], idf)
                    nc.vector.tensor_copy(
                        h1t.rearrange("p c (t f) -> p c t f", f=P)[:, :, it, :],
                        tp1.rearrange("p (c f) -> p c f", f=P))

                for dt in range(4):
                    for ih in range(2):
                        pp = preps.tile([P, 512], F32, name="pp", tag="pre")
                        for ct in range(4):
                            nc.tensor.matmul(
                                pp, w1[:, ct, dt * P:(dt + 1) * P],
                                h1t[:, ct, ih * 512:(ih + 1) * 512],
                                start=(ct == 0), stop=(ct == 3))
                        nc.scalar.activation(
                            out=qt_s[:, dt, ih * 512:(ih + 1) * 512], in_=pp,
                            func=AF.Identity, bias=bqs_pt[:, dt:dt + 1])

                for it in range(NIT):
                    pq = preps.tile([P, D], F32, name="pq", tag="pre")
                    for ct in range(4):
                        nc.tensor.matmul(pq, h1t[:, ct, it * P:(it + 1) * P],
                                         w2[:, ct, :], start=(ct == 0), stop=(ct == 3))
                    nc.vector.tensor_add(q_sb[:, it, :], pq, bq_bc)

            rid = nc.gpsimd.partition_id()

            o1_gate = {}
            # ================= main j-panel loop =================
            with (
                tc.tile_pool(name="work", bufs=1) as work,
                tc.tile_pool(name="psA", bufs=2, space="PSUM") as psA,   # attn (2x2 banks)
                tc.tile_pool(name="psT", bufs=1, space="PSUM") as psT,   # ET transp (2 banks)
                tc.tile_pool(name="psM", bufs=2, space="PSUM") as psM,   # mm2/mm3 (2 banks)
            ):
                for p in range(NPANELS):
                    j0 = p * PANEL
                    ch = next(k for k in range(NCHUNKS)
                              if CHUNK_BASE[k] <= j0 < CHUNK_BASE[k + 1])
                    ktp_t = []
                    for dt in range(4):
                        ktp = work.tile([P, PANEL], MM_DT, name="ktp", tag="ktp", bufs=5)
                        for jh in range(2):
                            nc.sync.dma_start(
                                out=ktp[:, jh * 512:(jh + 1) * 512],
                                in_=ag_out[p, dt * P:(dt + 1) * P,
                                           jh * 512:(jh + 1) * 512])
                        ktp_t.append(ktp)
                    ksb_t = []
                    for jt in range(NJT):
                        ksb = work.tile([P, D], MM_DT, name="ksb", tag="ksb", bufs=10)
                        nc.sync.dma_start(out=ksb, in_=ag2_out[p, jt * P:(jt + 1) * P, :])
                        ksb_t.append(ksb)

                    # attn matmuls + exp + attn output
                    e_t = []
                    for it in range(NIT):
                        pa = psA.tile([P, PANEL], F32, name="pa", tag="pa")
                        for dt in range(4):
                            for jh in range(2):
                                nc.tensor.matmul(
                                    pa[:, jh * 512:(jh + 1) * 512],
                                    qt_s[:, dt, it * P:(it + 1) * P],
                                    ktp_t[dt][:, jh * 512:(jh + 1) * 512],
                                    start=(dt == 0), stop=(dt == 3))
                        e = work.tile([P, PANEL], MM_DT, name="e", tag="e", bufs=9)
                        nc.scalar.activation(out=e, in_=pa, func=AF.Exp,
                                             accum_out=rowsum[:, it, p:p + 1])
                        e_t.append(e)
                        ao = work.tile([P, PANEL], F32, name="ao", tag="ao", bufs=3)
                        nc.vector.tensor_copy(ao, pa)
                        nc.sync.dma_start(
                            out=attn_o[it * P:(it + 1) * P, j0:j0 + PANEL], in_=ao)

                    # ET = E.T (PE transposes); colsum via ACT copy accum
                    et_t = []
                    for jt in range(NJT):
                        pt = psT.tile([P, NLOC], MM_DT, name="pt", tag="pt")
                        for it in range(NIT):
                            nc.tensor.transpose(pt[:, it * P:(it + 1) * P],
                                                e_t[it][:, jt * P:(jt + 1) * P], idr)
                        et = work.tile([P, NLOC], MM_DT, name="et", tag="et", bufs=9)
                        nc.scalar.activation(out=et, in_=pt, func=AF.Identity,
                                             accum_out=colsum[:, jt, p:p + 1])
                        et_t.append(et)

                    # mm2: P2[j, d] partials -> p2 bounce
                    for jt in range(NJT):
                        pm = psM.tile([P, D], F32, name="pm", tag="pm")
                        for it in range(NIT):
                            nc.tensor.matmul(pm, e_t[it][:, jt * P:(jt + 1) * P],
                                             q_sb[:, it, :], start=(it == 0),
                                             stop=(it == NIT - 1))
                        p2s = work.tile([P, D], F32, name="p2s", tag="p2s", bufs=4)
                        nc.vector.tensor_copy(p2s, pm)
                        r0 = j0 - CHUNK_BASE[ch] + jt * P
                        nc.sync.dma_start(out=p2b[ch][r0:r0 + P, 0:D], in_=p2s)

                    # mm3: out1[i, d] += E @ k (lhsT = ET)
                    for it in range(NIT):
                        pm = psM.tile([P, D], F32, name="pm3", tag="pm")
                        for jt in range(NJT):
                            nc.tensor.matmul(pm, et_t[jt][:, it * P:(it + 1) * P],
                                             ksb_t[jt], start=(jt == 0),
                                             stop=(jt == NJT - 1))
                        if p == 0:
                            o1_gate[p] = nc.vector.tensor_copy(o1_acc[:, it, :], pm)
                        else:
                            o1_gate[p] = nc.vector.tensor_add(
                                o1_acc[:, it, :], pm, o1_acc[:, it, :])

                    # colsum column for this panel
                    dst = bass.AP(
                        tensor=p2b[ch].tensor,
                        offset=p2b[ch].offset + (j0 - CHUNK_BASE[ch]) * WPAD + D,
                        ap=[[WPAD, P], [P * WPAD, NJT]])
                    nc.sync.dma_start(out=dst, in_=colsum[:, :, p])

                    if j0 + PANEL == CHUNK_BASE[ch + 1]:
                        nc.gpsimd.collective_compute(
                            "ReduceScatter", mybir.AluOpType.add,
                            replica_groups=[list(range(NCORES))],
                            ins=[p2b[ch][:].opt()], outs=[rs_out[ch][:].opt()])

                # ================= finalize =================
                # fused1 (DVE, core-local)
                rs_tot = work.tile([P, NIT], F32)
                nc.vector.tensor_reduce(rs_tot, rowsum, axis=mybir.AxisListType.X,
                                        op=mybir.AluOpType.add)
                rs_rec = work.tile([P, NIT], F32)
                nc.vector.reciprocal(rs_rec, rs_tot)
                for it in range(NIT):
                    f1s = work.tile([P, D], F32, name="f1s", tag="f1s", bufs=2)
                    nc.vector.tensor_scalar(
                        out=f1s, in0=o1_acc[:, it, :], scalar1=rs_rec[:, it:it + 1],
                        scalar2=None, op0=mybir.AluOpType.mult)
                    nc.vector.tensor_add(f1s, f1s, q_sb[:, it, :].bitcast(F32))
                    nc.sync.dma_start(out=f1_o[it * P:(it + 1) * P, :], in_=f1s)

                # fused2 per chunk (gpsimd only: don't stall DVE/PE on the RS)
                ag2_flat = ag2_out.rearrange("c j d -> (c j) d")
                out_r0 = 0
                from concourse.tile_rust import add_dep_helper

                fin_gate = {0: 5, 1: 7, 2: 7, 3: 7}
                for ck in range(NCHUNKS):
                    nb = CHUNK_PANELS[ck] * PANEL // NCORES // P  # band i-tiles
                    for t in range(nb):
                        rsb = work.tile([P, WPAD], F32, name="rsb", tag="rsb", bufs=3)
                        nc.gpsimd.dma_start(
                            out=rsb,
                            in_=rs_out[ck][t * P:(t + 1) * P, :])
                        km = work.tile([P, D], MM_DT, name="km", tag="km", bufs=3)
                        for r in range(NCORES):
                            g0 = CHUNK_BASE[ck] + (r * nb + t) * P
                            nc.gpsimd.dma_start(
                                out=km, in_=ag2_flat[g0:g0 + P, :], cond=(rid == r))
                        crec = work.tile([P, 1], F32, name="crec", tag="crec", bufs=3)
                        h = nc.vector.reciprocal(crec, rsb[:, D:D + 1])
                        add_dep_helper(h.ins, o1_gate[fin_gate[ck]].ins, False,
                                       "fused2 finalize after late panel")
                        f2s = work.tile([P, D], F32, name="f2s", tag="f2s", bufs=3)
                        nc.vector.tensor_scalar(
                            out=f2s, in0=rsb[:, 0:D], scalar1=crec,
                            scalar2=None, op0=ALU.mult)
                        nc.vector.tensor_add(f2s, f2s, km.bitcast(F32))
                        nc.sync.dma_start(
                            out=f2_o[out_r0:out_r0 + P, :], in_=f2s)
                        out_r0 += P

    nc.compile()
    return nc


def _get_nc():
    if "nc" not in _nc_cache:
        _nc_cache["nc"] = _build_nc()
    return _nc_cache["nc"]


def _make_in_maps(h1, h2, Wq, bq, Wk, bk):
    h1 = np.ascontiguousarray(h1, np.float32)
    h2 = np.ascontiguousarray(h2, np.float32)
    s = np.float32(1.0 / np.sqrt(D))
    wqt = np.ascontiguousarray(np.asarray(Wq, np.float32).T)
    in_common = {
        "wqt_s": wqt * s,
        "wqt": wqt,
        "wkt": np.ascontiguousarray(np.asarray(Wk, np.float32).T),
        "bq_s": (np.asarray(bq, np.float32) * s).reshape(1, D),
        "bq": np.asarray(bq, np.float32).reshape(1, D),
        "bk": np.asarray(bk, np.float32).reshape(1, D),
    }
    return [
        {"h1": h1[c * NLOC:(c + 1) * NLOC], "h2": h2[c * NLOC:(c + 1) * NLOC],
         **in_common}
        for c in range(NCORES)
    ]


def _assemble(res):
    attn = np.concatenate([r["attn"] for r in res], axis=0)
    fused1 = np.concatenate([r["f1"] for r in res], axis=0)
    fused2 = np.empty((M, D), np.float32)
    for c in range(NCORES):
        f2c = res[c]["f2"]
        o = 0
        for ck in range(NCHUNKS):
            nb = CHUNK_PANELS[ck] * PANEL // NCORES
            g0 = CHUNK_BASE[ck] + c * nb
            fused2[g0:g0 + nb] = f2c[o:o + nb]
            o += nb
    return fused1, fused2, attn


def kernel(h1, h2, Wq, bq, Wk, bk):
    from concourse.bass_utils import run_bass_kernel_spmd

    in_maps = _make_in_maps(h1, h2, Wq, bq, Wk, bk)
    nc = _get_nc()
    res = run_bass_kernel_spmd(nc, in_maps, core_ids=list(range(NCORES))).results
    return _assemble(res)


# revision 16
# speedup vs baseline: 1.3005x; 1.1379x over previous
"""ContrastiveAttentionCompensation on 8 TRN2 NeuronCores (Bass/Tile).

Reference computation (N = M = 8192, D = 512, fp32):
    q = h1 @ Wq.T + bq                  [N, D]
    k = h2 @ Wk.T + bk                  [M, D]
    attn = (q @ k.T) / sqrt(D)          [N, M]
    soft_text = softmax(attn, axis=-1)  row softmax
    soft_img  = softmax(attn, axis=0)   column softmax
    fused1 = soft_text @ k + q          [N, D]
    fused2 = soft_img.T @ q + k         [M, D]
    returns (fused1, fused2, attn)

Sharding: rows of h1 (N dim of the score matrix) across 8 cores. Each core
computes its [1024, 8192] slab of attn / E = exp(attn):
  - row softmax is core-local (full M per core); fused1 = (E@k)/row_sum + q.
  - fused2 needs sum over N of E[i,j] q[i,d] -> per-core partials P2[j,d] and
    column sums, reduced with chunked ReduceScatters (colsum rides as column
    512 of the 520-wide RS buffer).
  - the k projection is computed sharded as kT and AllGathered (f32r bits);
    k-natural panels are derived on-chip by PE transposes of the kT stream.
Softmax skips max subtraction (attn is O(6); exp is safe in fp32 and matches
jax.nn.softmax to fp32 accuracy).

Matmul dtype: float32r (1 cyc/row) by default; float32 would be 4 cyc/row.
"""
import sys

sys.path.insert(0, "/opt/trn_rl_repo")

import numpy as np

N, M, D = 8192, 8192, 512
NCORES = 8
NLOC = N // NCORES          # 1024 rows per core
P = 128
NIT = NLOC // P             # 8 i-tiles per core
PANEL = 1024                # j-panel width
NPANELS = M // PANEL        # 8
NJT = PANEL // P            # 8 j-tiles per panel
CHUNK_PANELS = (2, 2, 2, 1, 1)   # ReduceScatter chunking (panels per chunk)
NCHUNKS = len(CHUNK_PANELS)
CHUNK_BASE = [sum(CHUNK_PANELS[:k]) * PANEL for k in range(NCHUNKS + 1)]
BAND = [CHUNK_PANELS[k] * PANEL // NCORES for k in range(NCHUNKS)]  # rows/core
KOFF = [sum(BAND[:k]) for k in range(NCHUNKS + 1)]                  # f2 offsets
WPAD = 520                  # 512 d-cols + colsum col (512) + pad to 32B


def _chunk_of_panel(p):
    j0 = p * PANEL
    return next(k for k in range(NCHUNKS) if CHUNK_BASE[k] <= j0 < CHUNK_BASE[k + 1])


_nc_cache = {}


def _build_nc():
    import concourse.bass as bass
    import concourse.mybir as mybir
    import concourse.tile as tile
    from concourse import bacc
    from concourse.masks import make_identity
    from concourse.tile_rust import add_dep_helper

    F32 = mybir.dt.float32
    F32R = mybir.dt.float32r
    MM_DT = F32R
    AF = mybir.ActivationFunctionType
    ALU = mybir.AluOpType

    nc = bacc.Bacc(None, num_devices=NCORES)

    h1 = nc.declare_dram_parameter("h1", [NLOC, D], F32, isOutput=False)
    h2 = nc.declare_dram_parameter("h2", [NLOC, D], F32, isOutput=False)
    wqt_s = nc.declare_dram_parameter("wqt_s", [D, D], F32, isOutput=False)
    wqt = nc.declare_dram_parameter("wqt", [D, D], F32, isOutput=False)
    wkt = nc.declare_dram_parameter("wkt", [D, D], F32, isOutput=False)
    bq_s = nc.declare_dram_parameter("bq_s", [1, D], F32, isOutput=False)
    bq = nc.declare_dram_parameter("bq", [1, D], F32, isOutput=False)
    bk = nc.declare_dram_parameter("bk", [1, D], F32, isOutput=False)

    attn_o = nc.declare_dram_parameter("attn", [NLOC, M], F32, isOutput=True)
    f1_o = nc.declare_dram_parameter("f1", [NLOC, D], F32, isOutput=True)
    f2_o = nc.declare_dram_parameter("f2", [NLOC, D], F32, isOutput=True)

    def bcast_row(ap_1d, parts=P):
        return bass.AP(tensor=ap_1d.tensor, offset=ap_1d.offset,
                       ap=[[0, parts]] + ap_1d.ap[1:])

    with tile.TileContext(nc) as tc:
        with (
            tc.tile_pool(name="persist", bufs=1) as pers,
            tc.tile_pool(name="dram", bufs=1, space="DRAM") as dram,
        ):
            idr = pers.tile([P, P], MM_DT)
            idf = pers.tile([P, P], F32)
            make_identity(nc, idf)
            nc.vector.tensor_copy(idr, idf)

            qt_s = pers.tile([P, 4, NLOC], MM_DT)     # qT' [d, i] scaled+bias
            q_sb = pers.tile([P, NIT, D], MM_DT)      # q natural [i, d]
            o1_acc = pers.tile([P, NIT, D], F32)      # E @ k accumulator
            rowsum = pers.tile([P, NIT, NPANELS], F32)
            colsum = pers.tile([P, NJT, NPANELS], F32)
            bqs_pt = pers.tile([P, 4], F32)
            bk_pt = pers.tile([P, 4], F32)
            bq_bc = pers.tile([P, D], F32)

            nc.sync.dma_start(out=bqs_pt, in_=bq_s[0, :].rearrange("(t p) -> p t", p=P))
            nc.sync.dma_start(out=bk_pt, in_=bk[0, :].rearrange("(t p) -> p t", p=P))
            nc.sync.dma_start(out=bq_bc, in_=bcast_row(bq[0:1, :]))

            ag_in = dram.tile([D, NLOC], MM_DT)                       # kT shard
            ag_out = dram.tile([NCORES, D, NLOC], MM_DT, addr_space="Shared")
            km_d = dram.tile([NLOC, D], MM_DT)                        # my k rows
            p2b = [dram.tile([CHUNK_PANELS[k] * PANEL, WPAD], F32, name=f"p2b{k}")
                   for k in range(NCHUNKS)]
            rs_out = [dram.tile([BAND[k], WPAD], F32, name=f"rso{k}")
                      for k in range(NCHUNKS)]

            # ================= precompute =================
            with (
                tc.tile_pool(name="pre", bufs=1) as pre,
                tc.tile_pool(name="preps", bufs=4, space="PSUM") as preps,
            ):
                # --- h2 side first: get the AllGather in flight ASAP
                h2_sb = pre.tile([P, NIT, D], F32)
                w3 = pre.tile([P, 4, D], MM_DT, name="w3")
                w3f = pre.tile([P, 4, D], F32, name="w3f")
                h2r = h2[:, :].rearrange("(t p) d -> p t d", p=P)
                for it in range(NIT):
                    nc.sync.dma_start(out=h2_sb[:, it, :], in_=h2r[:, it, :])
                w3r = wkt[:, :].rearrange("(t p) d -> p t d", p=P)
                for ct in range(4):
                    nc.sync.dma_start(out=w3f[:, ct, :], in_=w3r[:, ct, :])
                nc.vector.tensor_copy(w3, w3f)

                h2t = pre.tile([P, 4, NLOC], MM_DT)
                for it in range(NIT):
                    tp2 = preps.tile([P, D], F32, name="tp2", tag="pre")
                    for ct in range(4):
                        nc.tensor.transpose(tp2[:, ct * P:(ct + 1) * P],
                                            h2_sb[:, it, ct * P:(ct + 1) * P], idf)
                    nc.vector.tensor_copy(
                        h2t.rearrange("p c (t f) -> p c t f", f=P)[:, :, it, :],
                        tp2.rearrange("p (c f) -> p c f", f=P))

                # kT shard [d, j_loc] -> AllGather
                kts = pre.tile([P, 4, NLOC], MM_DT)
                ag_in_r = ag_in[:, :].rearrange("(t p) j -> p t j", p=P)
                for dt in range(4):
                    for jh in range(2):
                        pk = preps.tile([P, 512], F32, name="pk", tag="pre")
                        for ct in range(4):
                            nc.tensor.matmul(
                                pk, w3[:, ct, dt * P:(dt + 1) * P],
                                h2t[:, ct, jh * 512:(jh + 1) * 512],
                                start=(ct == 0), stop=(ct == 3))
                        nc.scalar.activation(
                            out=kts[:, dt, jh * 512:(jh + 1) * 512], in_=pk,
                            func=AF.Identity, bias=bk_pt[:, dt:dt + 1])
                        nc.sync.dma_start(
                            out=ag_in_r[:, dt, jh * 512:(jh + 1) * 512],
                            in_=kts[:, dt, jh * 512:(jh + 1) * 512])
                nc.gpsimd.collective_compute(
                    "AllGather", mybir.AluOpType.bypass,
                    replica_groups=[list(range(NCORES))],
                    ins=[ag_in[:].opt()], outs=[ag_out[:].opt()])

                # --- h1 side (overlaps the AllGather)
                h1_sb = pre.tile([P, NIT, D], F32)
                w1 = pre.tile([P, 4, D], MM_DT, name="w1")
                w2 = pre.tile([P, 4, D], MM_DT, name="w2")
                w1f = pre.tile([P, 4, D], F32, name="w1f")
                w2f = pre.tile([P, 4, D], F32, name="w2f")
                h1r = h1[:, :].rearrange("(t p) d -> p t d", p=P)
                for it in range(NIT):
                    nc.sync.dma_start(out=h1_sb[:, it, :], in_=h1r[:, it, :])
                for wf, wr, src in ((w1f, w1, wqt_s), (w2f, w2, wqt)):
                    wsr = src[:, :].rearrange("(t p) d -> p t d", p=P)
                    for ct in range(4):
                        nc.sync.dma_start(out=wf[:, ct, :], in_=wsr[:, ct, :])
                    nc.vector.tensor_copy(wr, wf)

                h1t = pre.tile([P, 4, NLOC], MM_DT)
                for it in range(NIT):
                    tp1 = preps.tile([P, D], F32, name="tp1", tag="pre")
                    for ct in range(4):
                        nc.tensor.transpose(tp1[:, ct * P:(ct + 1) * P],
                                            h1_sb[:, it, ct * P:(ct + 1) * P], idf)
                    nc.vector.tensor_copy(
                        h1t.rearrange("p c (t f) -> p c t f", f=P)[:, :, it, :],
                        tp1.rearrange("p (c f) -> p c f", f=P))

                for dt in range(4):
                    for ih in range(2):
                        pp = preps.tile([P, 512], F32, name="pp", tag="pre")
                        for ct in range(4):
                            nc.tensor.matmul(
                                pp, w1[:, ct, dt * P:(dt + 1) * P],
                                h1t[:, ct, ih * 512:(ih + 1) * 512],
                                start=(ct == 0), stop=(ct == 3))
                        nc.scalar.activation(
                            out=qt_s[:, dt, ih * 512:(ih + 1) * 512], in_=pp,
                            func=AF.Identity, bias=bqs_pt[:, dt:dt + 1])

                for it in range(NIT):
                    pq = preps.tile([P, D], F32, name="pq", tag="pre")
                    for ct in range(4):
                        nc.tensor.matmul(pq, h1t[:, ct, it * P:(it + 1) * P],
                                         w2[:, ct, :], start=(ct == 0), stop=(ct == 3))
                    nc.vector.tensor_add(q_sb[:, it, :], pq, bq_bc)

            rid = nc.sync.partition_id()

            o1_gate = {}
            # ================= main j-panel loop =================
            with (
                tc.tile_pool(name="work", bufs=1) as work,
                tc.tile_pool(name="psA", bufs=2, space="PSUM") as psA,   # attn 2x2 banks
                tc.tile_pool(name="psT", bufs=2, space="PSUM") as psT,   # transposes
                tc.tile_pool(name="psM", bufs=2, space="PSUM") as psM,   # mm2/mm3
            ):
                for p in range(NPANELS):
                    j0 = p * PANEL
                    ch = _chunk_of_panel(p)
                    ktp_t = []
                    for dt in range(4):
                        ktp = work.tile([P, PANEL], MM_DT, name="ktp", tag="ktp", bufs=5)
                        for jh in range(2):
                            nc.sync.dma_start(
                                out=ktp[:, jh * 512:(jh + 1) * 512],
                                in_=ag_out[p, dt * P:(dt + 1) * P,
                                           jh * 512:(jh + 1) * 512])
                        ktp_t.append(ktp)

                    # attn matmuls + exp + attn output
                    e_t = []
                    for it in range(NIT):
                        pa = psA.tile([P, PANEL], F32, name="pa", tag="pa")
                        for dt in range(4):
                            for jh in range(2):
                                nc.tensor.matmul(
                                    pa[:, jh * 512:(jh + 1) * 512],
                                    qt_s[:, dt, it * P:(it + 1) * P],
                                    ktp_t[dt][:, jh * 512:(jh + 1) * 512],
                                    start=(dt == 0), stop=(dt == 3))
                        e = work.tile([P, PANEL], MM_DT, name="e", tag="e", bufs=9)
                        nc.scalar.activation(out=e, in_=pa, func=AF.Exp,
                                             accum_out=rowsum[:, it, p:p + 1])
                        e_t.append(e)
                        ao = work.tile([P, PANEL], F32, name="ao", tag="ao", bufs=3)
                        nc.vector.tensor_copy(ao, pa)
                        nc.sync.dma_start(
                            out=attn_o[it * P:(it + 1) * P, j0:j0 + PANEL], in_=ao)

                    # k natural panels via PE transposes of kT
                    ksb_t = []
                    for jt in range(NJT):
                        pkt = psT.tile([P, D], MM_DT, name="pkt", tag="pt")
                        for dt in range(4):
                            nc.tensor.transpose(
                                pkt[:, dt * P:(dt + 1) * P],
                                ktp_t[dt][:, jt * P:(jt + 1) * P], idr)
                        ksb = work.tile([P, D], MM_DT, name="ksb", tag="ksb", bufs=10)
                        nc.vector.tensor_copy(ksb, pkt)
                        ksb_t.append(ksb)
                        # stash my fused2 band's k rows into DRAM scratch
                        gj = j0 + jt * P
                        owner = (gj - CHUNK_BASE[ch]) // BAND[ch]
                        krow = KOFF[ch] + (gj - CHUNK_BASE[ch]) - owner * BAND[ch]
                        nc.sync.dma_start(out=km_d[krow:krow + P, :], in_=ksb,
                                          cond=(rid == owner))

                    # ET = E.T (PE transposes); colsum halves via ACT copy accum
                    cs2 = work.tile([P, NJT, 2], F32, name="cs2", tag="cs2", bufs=2)
                    et_t = []
                    for jt in range(NJT):
                        et = work.tile([P, NLOC], MM_DT, name="et", tag="et", bufs=9)
                        for ih in range(2):
                            pt = psT.tile([P, D], MM_DT, name="pt", tag="pt")
                            for it2 in range(4):
                                it = ih * 4 + it2
                                nc.tensor.transpose(pt[:, it2 * P:(it2 + 1) * P],
                                                    e_t[it][:, jt * P:(jt + 1) * P], idr)
                            nc.scalar.activation(
                                out=et[:, ih * D:(ih + 1) * D], in_=pt,
                                func=AF.Identity, accum_out=cs2[:, jt, ih:ih + 1])
                        et_t.append(et)
                    nc.vector.tensor_add(colsum[:, :, p], cs2[:, :, 0], cs2[:, :, 1])

                    # mm2: P2[j, d] partials -> p2 bounce
                    for jt in range(NJT):
                        pm = psM.tile([P, D], F32, name="pm", tag="pm")
                        for it in range(NIT):
                            nc.tensor.matmul(pm, e_t[it][:, jt * P:(jt + 1) * P],
                                             q_sb[:, it, :], start=(it == 0),
                                             stop=(it == NIT - 1))
                        p2s = work.tile([P, D], F32, name="p2s", tag="p2s", bufs=3)
                        nc.vector.tensor_copy(p2s, pm)
                        r0 = j0 - CHUNK_BASE[ch] + jt * P
                        nc.sync.dma_start(out=p2b[ch][r0:r0 + P, 0:D], in_=p2s)

                    # mm3: out1[i, d] += E @ k (lhsT = ET)
                    for it in range(NIT):
                        pm = psM.tile([P, D], F32, name="pm3", tag="pm")
                        for jt in range(NJT):
                            nc.tensor.matmul(pm, et_t[jt][:, it * P:(it + 1) * P],
                                             ksb_t[jt], start=(jt == 0),
                                             stop=(jt == NJT - 1))
                        if p == 0:
                            o1_gate[p] = nc.vector.tensor_copy(o1_acc[:, it, :], pm)
                        else:
                            o1_gate[p] = nc.vector.tensor_add(
                                o1_acc[:, it, :], pm, o1_acc[:, it, :])

                    # colsum column for this panel
                    dst = bass.AP(
                        tensor=p2b[ch].tensor,
                        offset=p2b[ch].offset + (j0 - CHUNK_BASE[ch]) * WPAD + D,
                        ap=[[WPAD, P], [P * WPAD, NJT]])
                    nc.sync.dma_start(out=dst, in_=colsum[:, :, p])

                    if j0 + PANEL == CHUNK_BASE[ch + 1]:
                        nc.gpsimd.collective_compute(
                            "ReduceScatter", mybir.AluOpType.add,
                            replica_groups=[list(range(NCORES))],
                            ins=[p2b[ch][:].opt()], outs=[rs_out[ch][:].opt()])

                # ================= finalize =================
                rs_tot = work.tile([P, NIT], F32)
                nc.vector.tensor_reduce(rs_tot, rowsum, axis=mybir.AxisListType.X,
                                        op=mybir.AluOpType.add)
                rs_rec = work.tile([P, NIT], F32)
                nc.vector.reciprocal(rs_rec, rs_tot)
                for it in range(NIT):
                    f1s = work.tile([P, D], F32, name="f1s", tag="f1s", bufs=2)
                    nc.vector.tensor_scalar(
                        out=f1s, in0=o1_acc[:, it, :], scalar1=rs_rec[:, it:it + 1],
                        scalar2=None, op0=mybir.AluOpType.mult)
                    nc.vector.tensor_add(f1s, f1s, q_sb[:, it, :].bitcast(F32))
                    nc.sync.dma_start(out=f1_o[it * P:(it + 1) * P, :], in_=f1s)

                fin_gate = {0: 4, 1: 6, 2: 7, 3: 7, 4: 7}
                for ck in range(NCHUNKS):
                    nb = BAND[ck] // P
                    for t in range(nb):
                        rsb = work.tile([P, WPAD], F32, name="rsb", tag="rsb", bufs=2)
                        nc.gpsimd.dma_start(
                            out=rsb, in_=rs_out[ck][t * P:(t + 1) * P, :])
                        km = work.tile([P, D], MM_DT, name="km", tag="km", bufs=2)
                        nc.gpsimd.dma_start(
                            out=km, in_=km_d[KOFF[ck] + t * P:KOFF[ck] + (t + 1) * P, :])
                        crec = work.tile([P, 1], F32, name="crec", tag="crec", bufs=2)
                        h = nc.vector.reciprocal(crec, rsb[:, D:D + 1])
                        add_dep_helper(h.ins, o1_gate[fin_gate[ck]].ins, False,
                                       "fused2 finalize after late panel")
                        f2s = work.tile([P, D], F32, name="f2s", tag="f2s", bufs=2)
                        nc.vector.tensor_scalar(
                            out=f2s, in0=rsb[:, 0:D], scalar1=crec,
                            scalar2=None, op0=ALU.mult)
                        nc.vector.tensor_add(f2s, f2s, km.bitcast(F32))
                        r0 = KOFF[ck] + t * P
                        nc.sync.dma_start(out=f2_o[r0:r0 + P, :], in_=f2s)

    nc.compile()
    return nc


def _get_nc():
    if "nc" not in _nc_cache:
        _nc_cache["nc"] = _build_nc()
    return _nc_cache["nc"]


def _make_in_maps(h1, h2, Wq, bq, Wk, bk):
    h1 = np.ascontiguousarray(h1, np.float32)
    h2 = np.ascontiguousarray(h2, np.float32)
    s = np.float32(1.0 / np.sqrt(D))
    wqt = np.ascontiguousarray(np.asarray(Wq, np.float32).T)
    in_common = {
        "wqt_s": wqt * s,
        "wqt": wqt,
        "wkt": np.ascontiguousarray(np.asarray(Wk, np.float32).T),
        "bq_s": (np.asarray(bq, np.float32) * s).reshape(1, D),
        "bq": np.asarray(bq, np.float32).reshape(1, D),
        "bk": np.asarray(bk, np.float32).reshape(1, D),
    }
    return [
        {"h1": h1[c * NLOC:(c + 1) * NLOC], "h2": h2[c * NLOC:(c + 1) * NLOC],
         **in_common}
        for c in range(NCORES)
    ]


def _assemble(res):
    attn = np.concatenate([r["attn"] for r in res], axis=0)
    fused1 = np.concatenate([r["f1"] for r in res], axis=0)
    fused2 = np.empty((M, D), np.float32)
    for c in range(NCORES):
        f2c = res[c]["f2"]
        for ck in range(NCHUNKS):
            g0 = CHUNK_BASE[ck] + c * BAND[ck]
            fused2[g0:g0 + BAND[ck]] = f2c[KOFF[ck]:KOFF[ck] + BAND[ck]]
    return fused1, fused2, attn


def kernel(h1, h2, Wq, bq, Wk, bk):
    from concourse.bass_utils import run_bass_kernel_spmd

    in_maps = _make_in_maps(h1, h2, Wq, bq, Wk, bk)
    nc = _get_nc()
    res = run_bass_kernel_spmd(nc, in_maps, core_ids=list(range(NCORES))).results
    return _assemble(res)


# revision 23
# speedup vs baseline: 1.4046x; 1.0801x over previous
"""ContrastiveAttentionCompensation on 8 TRN2 NeuronCores (Bass/Tile).

Reference computation (N = M = 8192, D = 512, fp32):
    q = h1 @ Wq.T + bq                  [N, D]
    k = h2 @ Wk.T + bk                  [M, D]
    attn = (q @ k.T) / sqrt(D)          [N, M]
    soft_text = softmax(attn, axis=-1)  row softmax
    soft_img  = softmax(attn, axis=0)   column softmax
    fused1 = soft_text @ k + q          [N, D]
    fused2 = soft_img.T @ q + k         [M, D]
    returns (fused1, fused2, attn)

Sharding: rows of h1 (N dim of the score matrix) across 8 cores. Each core
computes its [1024, 8192] slab of attn / E = exp(attn):
  - row softmax is core-local (full M per core); fused1 = (E@k)/row_sum + q.
  - fused2 needs sum over N of E[i,j] q[i,d] -> per-core partials P2[j,d] and
    column sums, reduced with chunked ReduceScatters (colsum rides as column
    512 of the 520-wide RS buffer).
  - the k projection is computed sharded as kT and AllGathered (f32r bits);
    k-natural panels are derived on-chip by PE transposes of the kT stream.
Softmax skips max subtraction (attn is O(6); exp is safe in fp32 and matches
jax.nn.softmax to fp32 accuracy).

Matmul dtype: float32r (1 cyc/row) by default; float32 would be 4 cyc/row.
"""
import sys

sys.path.insert(0, "/opt/trn_rl_repo")

import numpy as np

N, M, D = 8192, 8192, 512
NCORES = 8
NLOC = N // NCORES          # 1024 rows per core
P = 128
NIT = NLOC // P             # 8 i-tiles per core
PANEL = 1024                # j-panel width
NPANELS = M // PANEL        # 8
NJT = PANEL // P            # 8 j-tiles per panel
CHUNK_PANELS = (2, 2, 2, 1, 1)   # ReduceScatter chunking (panels per chunk)
NCHUNKS = len(CHUNK_PANELS)
CHUNK_BASE = [sum(CHUNK_PANELS[:k]) * PANEL for k in range(NCHUNKS + 1)]
BAND = [CHUNK_PANELS[k] * PANEL // NCORES for k in range(NCHUNKS)]  # rows/core
KOFF = [sum(BAND[:k]) for k in range(NCHUNKS + 1)]                  # f2 offsets
WPAD = 520                  # 512 d-cols + colsum col (512) + pad to 32B


def _chunk_of_panel(p):
    j0 = p * PANEL
    return next(k for k in range(NCHUNKS) if CHUNK_BASE[k] <= j0 < CHUNK_BASE[k + 1])


_nc_cache = {}


def _build_nc():
    import concourse.bass as bass
    import concourse.mybir as mybir
    import concourse.tile as tile
    from concourse import bacc
    from concourse.masks import make_identity
    from concourse.tile_rust import add_dep_helper

    F32 = mybir.dt.float32
    F32R = mybir.dt.float32r
    BF16 = mybir.dt.bfloat16
    MM_DT = F32R
    E_DT = BF16
    AF = mybir.ActivationFunctionType
    ALU = mybir.AluOpType

    nc = bacc.Bacc(None, num_devices=NCORES)

    h1 = nc.declare_dram_parameter("h1", [NLOC, D], F32, isOutput=False)
    h2 = nc.declare_dram_parameter("h2", [NLOC, D], F32, isOutput=False)
    h2h = nc.declare_dram_parameter("h2h", [PANEL, D], F32, isOutput=False)
    wqt_s = nc.declare_dram_parameter("wqt_s", [D, D], F32, isOutput=False)
    wqt = nc.declare_dram_parameter("wqt", [D, D], F32, isOutput=False)
    wkt = nc.declare_dram_parameter("wkt", [D, D], F32, isOutput=False)
    bq_s = nc.declare_dram_parameter("bq_s", [1, D], F32, isOutput=False)
    bq = nc.declare_dram_parameter("bq", [1, D], F32, isOutput=False)
    bk = nc.declare_dram_parameter("bk", [1, D], F32, isOutput=False)

    attn_o = nc.declare_dram_parameter("attn", [NLOC, M], F32, isOutput=True)
    f1_o = nc.declare_dram_parameter("f1", [NLOC, D], F32, isOutput=True)
    f2_o = nc.declare_dram_parameter("f2", [NLOC, D], F32, isOutput=True)

    def bcast_row(ap_1d, parts=P):
        return bass.AP(tensor=ap_1d.tensor, offset=ap_1d.offset,
                       ap=[[0, parts]] + ap_1d.ap[1:])

    with tile.TileContext(nc) as tc:
        with (
            tc.tile_pool(name="persist", bufs=1) as pers,
            tc.tile_pool(name="dram", bufs=1, space="DRAM") as dram,
        ):
            idr = pers.tile([P, P], MM_DT)
            idb = pers.tile([P, P], E_DT)
            idf = pers.tile([P, P], F32)
            make_identity(nc, idf)
            nc.vector.tensor_copy(idr, idf)
            nc.vector.tensor_copy(idb, idf)

            qt_s = pers.tile([P, 4, NLOC], MM_DT)     # qT' [d, i] scaled+bias
            q_sb = pers.tile([P, NIT, D], MM_DT)      # q natural [i, d]
            q_bf = pers.tile([P, NIT, D], E_DT)       # q for mm2 (bf16)
            kth = pers.tile([P, 4, PANEL], MM_DT)     # panel-0 kT (replicated)
            o1_acc = pers.tile([P, NIT, D], F32)      # E @ k accumulator
            rowsum = pers.tile([P, NIT, NPANELS], F32)
            colsum = pers.tile([P, NJT, NPANELS], F32)
            bqs_pt = pers.tile([P, 4], F32)
            bk_pt = pers.tile([P, 4], F32)
            bq_bc = pers.tile([P, D], F32)

            nc.sync.dma_start(out=bqs_pt, in_=bq_s[0, :].rearrange("(t p) -> p t", p=P))
            nc.sync.dma_start(out=bk_pt, in_=bk[0, :].rearrange("(t p) -> p t", p=P))
            nc.sync.dma_start(out=bq_bc, in_=bcast_row(bq[0:1, :]))

            ag_in = dram.tile([D, NLOC], MM_DT)                       # kT shard
            ag_out = dram.tile([NCORES, D, NLOC], MM_DT, addr_space="Shared")
            km_d = dram.tile([NLOC, D], E_DT)                        # my k rows
            p2b = [dram.tile([CHUNK_PANELS[k] * PANEL, WPAD], F32, name=f"p2b{k}")
                   for k in range(NCHUNKS)]
            rs_out = [dram.tile([BAND[k], WPAD], F32, name=f"rso{k}")
                      for k in range(NCHUNKS)]

            # ================= precompute =================
            with (
                tc.tile_pool(name="pre", bufs=1) as pre,
                tc.tile_pool(name="preps", bufs=4, space="PSUM") as preps,
            ):
                # --- h2 side first: get the AllGather in flight ASAP
                h2_sb = pre.tile([P, NIT, D], F32, tag="hbuf", bufs=2)
                w3 = pre.tile([P, 4, D], MM_DT, name="w3")
                w3f = pre.tile([P, 4, D], F32, name="w3f", tag="wf", bufs=1)
                h2r = h2[:, :].rearrange("(t p) d -> p t d", p=P)
                for it in range(NIT):
                    nc.sync.dma_start(out=h2_sb[:, it, :], in_=h2r[:, it, :])
                w3r = wkt[:, :].rearrange("(t p) d -> p t d", p=P)
                for ct in range(4):
                    nc.sync.dma_start(out=w3f[:, ct, :], in_=w3r[:, ct, :])
                nc.vector.tensor_copy(w3, w3f)

                h2t = pre.tile([P, 4, NLOC], MM_DT, tag="ht", bufs=2)
                for it in range(NIT):
                    tp2 = preps.tile([P, D], F32, name="tp2", tag="pre")
                    for ct in range(4):
                        nc.tensor.transpose(tp2[:, ct * P:(ct + 1) * P],
                                            h2_sb[:, it, ct * P:(ct + 1) * P], idf)
                    nc.vector.tensor_copy(
                        h2t.rearrange("p c (t f) -> p c t f", f=P)[:, :, it, :],
                        tp2.rearrange("p (c f) -> p c f", f=P))

                # kT shard [d, j_loc] -> AllGather
                kts = pre.tile([P, 4, NLOC], MM_DT, tag="kh1", bufs=2)
                ag_in_r = ag_in[:, :].rearrange("(t p) j -> p t j", p=P)
                for dt in range(4):
                    for jh in range(2):
                        pk = preps.tile([P, 512], F32, name="pk", tag="pre")
                        for ct in range(4):
                            nc.tensor.matmul(
                                pk, w3[:, ct, dt * P:(dt + 1) * P],
                                h2t[:, ct, jh * 512:(jh + 1) * 512],
                                start=(ct == 0), stop=(ct == 3))
                        nc.scalar.activation(
                            out=kts[:, dt, jh * 512:(jh + 1) * 512], in_=pk,
                            func=AF.Identity, bias=bk_pt[:, dt:dt + 1])
                        nc.sync.dma_start(
                            out=ag_in_r[:, dt, jh * 512:(jh + 1) * 512],
                            in_=kts[:, dt, jh * 512:(jh + 1) * 512])
                nc.gpsimd.collective_compute(
                    "AllGather", mybir.AluOpType.bypass,
                    replica_groups=[list(range(NCORES))],
                    ins=[ag_in[:].opt()], outs=[ag_out[:].opt()])

                # --- h1 side (overlaps the AllGather)
                h1_sb = pre.tile([P, NIT, D], F32, tag="hbuf", bufs=2)
                w1 = pre.tile([P, 4, D], MM_DT, name="w1")
                w2 = pre.tile([P, 4, D], MM_DT, name="w2")
                w1f = pre.tile([P, 4, D], F32, name="w1f", tag="wf", bufs=1)
                w2f = pre.tile([P, 4, D], F32, name="w2f", tag="wf", bufs=1)
                h1r = h1[:, :].rearrange("(t p) d -> p t d", p=P)
                for it in range(NIT):
                    nc.sync.dma_start(out=h1_sb[:, it, :], in_=h1r[:, it, :])
                for wf, wr, src in ((w1f, w1, wqt_s), (w2f, w2, wqt)):
                    wsr = src[:, :].rearrange("(t p) d -> p t d", p=P)
                    for ct in range(4):
                        nc.sync.dma_start(out=wf[:, ct, :], in_=wsr[:, ct, :])
                    nc.vector.tensor_copy(wr, wf)

                h1t = pre.tile([P, 4, NLOC], MM_DT, tag="kh1", bufs=2)
                for it in range(NIT):
                    tp1 = preps.tile([P, D], F32, name="tp1", tag="pre")
                    for ct in range(4):
                        nc.tensor.transpose(tp1[:, ct * P:(ct + 1) * P],
                                            h1_sb[:, it, ct * P:(ct + 1) * P], idf)
                    nc.vector.tensor_copy(
                        h1t.rearrange("p c (t f) -> p c t f", f=P)[:, :, it, :],
                        tp1.rearrange("p (c f) -> p c f", f=P))

                for dt in range(4):
                    for ih in range(2):
                        pp = preps.tile([P, 512], F32, name="pp", tag="pre")
                        for ct in range(4):
                            nc.tensor.matmul(
                                pp, w1[:, ct, dt * P:(dt + 1) * P],
                                h1t[:, ct, ih * 512:(ih + 1) * 512],
                                start=(ct == 0), stop=(ct == 3))
                        nc.scalar.activation(
                            out=qt_s[:, dt, ih * 512:(ih + 1) * 512], in_=pp,
                            func=AF.Identity, bias=bqs_pt[:, dt:dt + 1])

                for it in range(NIT):
                    pq = preps.tile([P, D], F32, name="pq", tag="pre")
                    for ct in range(4):
                        nc.tensor.matmul(pq, h1t[:, ct, it * P:(it + 1) * P],
                                         w2[:, ct, :], start=(ct == 0), stop=(ct == 3))
                    nc.vector.tensor_add(q_sb[:, it, :], pq, bq_bc)
                    nc.vector.tensor_copy(q_bf[:, it, :], q_sb[:, it, :])

                # replicated panel-0 kT head (hides the AllGather latency)
                h2h_sb = pre.tile([P, NIT, D], F32, tag="hbuf", bufs=2)
                h2hr = h2h[:, :].rearrange("(t p) d -> p t d", p=P)
                for it in range(NIT):
                    nc.sync.dma_start(out=h2h_sb[:, it, :], in_=h2hr[:, it, :])
                h2ht = pre.tile([P, 4, PANEL], MM_DT, tag="ht", bufs=2)
                for it in range(NIT):
                    tph = preps.tile([P, D], F32, name="tph", tag="pre")
                    for ct in range(4):
                        nc.tensor.transpose(tph[:, ct * P:(ct + 1) * P],
                                            h2h_sb[:, it, ct * P:(ct + 1) * P], idf)
                    nc.vector.tensor_copy(
                        h2ht.rearrange("p c (t f) -> p c t f", f=P)[:, :, it, :],
                        tph.rearrange("p (c f) -> p c f", f=P))
                for dt in range(4):
                    for jh in range(2):
                        pkh = preps.tile([P, 512], F32, name="pkh", tag="pre")
                        for ct in range(4):
                            nc.tensor.matmul(
                                pkh, w3[:, ct, dt * P:(dt + 1) * P],
                                h2ht[:, ct, jh * 512:(jh + 1) * 512],
                                start=(ct == 0), stop=(ct == 3))
                        nc.scalar.activation(
                            out=kth[:, dt, jh * 512:(jh + 1) * 512], in_=pkh,
                            func=AF.Identity, bias=bk_pt[:, dt:dt + 1])

            rid = nc.sync.partition_id()

            o1_gate = {}
            # ================= main j-panel loop =================
            with (
                tc.tile_pool(name="work", bufs=1) as work,
                tc.tile_pool(name="psA", bufs=2, space="PSUM") as psA,   # attn 2x2 banks
                tc.tile_pool(name="psT", bufs=1, space="PSUM") as psT,   # ET transposes
                tc.tile_pool(name="psK", bufs=1, space="PSUM") as psK,   # k transposes
                tc.tile_pool(name="psM", bufs=2, space="PSUM") as psM,   # mm2/mm3
            ):
                for p in range(NPANELS):
                    j0 = p * PANEL
                    ch = _chunk_of_panel(p)
                    if p == 0:
                        ktp_t = [kth[:, dt, :] for dt in range(4)]
                    else:
                        ktp_t = []
                        for dt in range(4):
                            ktp = work.tile([P, PANEL], MM_DT, name="ktp",
                                            tag="ktp", bufs=6)
                            for jh in range(2):
                                nc.sync.dma_start(
                                    out=ktp[:, jh * 512:(jh + 1) * 512],
                                    in_=ag_out[p, dt * P:(dt + 1) * P,
                                               jh * 512:(jh + 1) * 512])
                            ktp_t.append(ktp)

                    # attn matmuls + exp + attn output
                    e_t = []
                    for it in range(NIT):
                        pa = psA.tile([P, PANEL], F32, name="pa", tag="pa")
                        for dt in range(4):
                            for jh in range(2):
                                nc.tensor.matmul(
                                    pa[:, jh * 512:(jh + 1) * 512],
                                    qt_s[:, dt, it * P:(it + 1) * P],
                                    ktp_t[dt][:, jh * 512:(jh + 1) * 512],
                                    start=(dt == 0), stop=(dt == 3))
                        e = work.tile([P, PANEL], E_DT, name="e", tag="e", bufs=12)
                        nc.scalar.activation(out=e, in_=pa, func=AF.Exp,
                                             accum_out=rowsum[:, it, p:p + 1])
                        e_t.append(e)
                        ao = work.tile([P, PANEL], F32, name="ao", tag="ao", bufs=3)
                        nc.scalar.copy(ao, pa)
                        nc.sync.dma_start(
                            out=attn_o[it * P:(it + 1) * P, j0:j0 + PANEL], in_=ao)

                    # k natural panels via PE transposes of kT
                    ksb_t = []
                    for jt in range(NJT):
                        pkt = psK.tile([P, D], MM_DT, name="pkt", tag="pkt")
                        for dt in range(4):
                            nc.tensor.transpose(
                                pkt[:, dt * P:(dt + 1) * P],
                                ktp_t[dt][:, jt * P:(jt + 1) * P], idr)
                        ksb = work.tile([P, D], E_DT, name="ksb", tag="ksb", bufs=12)
                        nc.vector.tensor_copy(ksb, pkt)
                        ksb_t.append(ksb)
                        # stash my fused2 band's k rows into DRAM scratch
                        gj = j0 + jt * P
                        owner = (gj - CHUNK_BASE[ch]) // BAND[ch]
                        krow = KOFF[ch] + (gj - CHUNK_BASE[ch]) - owner * BAND[ch]
                        nc.sync.dma_start(out=km_d[krow:krow + P, :], in_=ksb,
                                          cond=(rid == owner))

                    # ET = E.T (PE transposes); colsum via ACT copy accum
                    et_t = []
                    for jt in range(NJT):
                        et = work.tile([P, NLOC], E_DT, name="et", tag="et", bufs=12)
                        pt = psT.tile([P, NLOC], E_DT, name="pt", tag="pt")
                        for it in range(NIT):
                            nc.tensor.transpose(pt[:, it * P:(it + 1) * P],
                                                e_t[it][:, jt * P:(jt + 1) * P], idb)
                        nc.scalar.activation(
                            out=et, in_=pt,
                            func=AF.Identity, accum_out=colsum[:, jt, p:p + 1])
                        et_t.append(et)

                    # mm2: P2[j, d] partials -> p2 bounce
                    for jt in range(NJT):
                        pm = psM.tile([P, D], F32, name="pm", tag="pm")
                        for it in range(NIT):
                            nc.tensor.matmul(pm, e_t[it][:, jt * P:(jt + 1) * P],
                                             q_bf[:, it, :], start=(it == 0),
                                             stop=(it == NIT - 1))
                        p2s = work.tile([P, D], F32, name="p2s", tag="p2s", bufs=3)
                        nc.vector.tensor_copy(p2s, pm)
                        r0 = j0 - CHUNK_BASE[ch] + jt * P
                        nc.sync.dma_start(out=p2b[ch][r0:r0 + P, 0:D], in_=p2s)

                    # mm3: out1[i, d] += E @ k (lhsT = ET)
                    for it in range(NIT):
                        pm = psM.tile([P, D], F32, name="pm3", tag="pm")
                        for jt in range(NJT):
                            nc.tensor.matmul(pm, et_t[jt][:, it * P:(it + 1) * P],
                                             ksb_t[jt], start=(jt == 0),
                                             stop=(jt == NJT - 1))
                        if p == 0:
                            o1_gate[p] = nc.vector.tensor_copy(o1_acc[:, it, :], pm)
                        else:
                            o1_gate[p] = nc.vector.tensor_add(
                                o1_acc[:, it, :], pm, o1_acc[:, it, :])

                    # colsum column for this panel
                    dst = bass.AP(
                        tensor=p2b[ch].tensor,
                        offset=p2b[ch].offset + (j0 - CHUNK_BASE[ch]) * WPAD + D,
                        ap=[[WPAD, P], [P * WPAD, NJT]])
                    nc.sync.dma_start(out=dst, in_=colsum[:, :, p])

                    if j0 + PANEL == CHUNK_BASE[ch + 1]:
                        nc.gpsimd.collective_compute(
                            "ReduceScatter", mybir.AluOpType.add,
                            replica_groups=[list(range(NCORES))],
                            ins=[p2b[ch][:].opt()], outs=[rs_out[ch][:].opt()])

                # ================= finalize =================
                rs_tot = work.tile([P, NIT], F32)
                nc.vector.tensor_reduce(rs_tot, rowsum, axis=mybir.AxisListType.X,
                                        op=mybir.AluOpType.add)
                rs_rec = work.tile([P, NIT], F32)
                nc.vector.reciprocal(rs_rec, rs_tot)
                for it in range(NIT):
                    f1s = work.tile([P, D], F32, name="f1s", tag="f1s", bufs=2)
                    nc.vector.tensor_scalar(
                        out=f1s, in0=o1_acc[:, it, :], scalar1=rs_rec[:, it:it + 1],
                        scalar2=None, op0=mybir.AluOpType.mult)
                    nc.vector.tensor_add(f1s, f1s, q_sb[:, it, :].bitcast(F32))
                    nc.sync.dma_start(out=f1_o[it * P:(it + 1) * P, :], in_=f1s)

                fin_gate = {0: 4, 1: 6, 2: 7, 3: 7, 4: 7}
                for ck in range(NCHUNKS):
                    nb = BAND[ck] // P
                    for t in range(nb):
                        rsb = work.tile([P, WPAD], F32, name="rsb", tag="rsb", bufs=2)
                        nc.gpsimd.dma_start(
                            out=rsb, in_=rs_out[ck][t * P:(t + 1) * P, :])
                        km = work.tile([P, D], E_DT, name="km", tag="km", bufs=2)
                        nc.gpsimd.dma_start(
                            out=km, in_=km_d[KOFF[ck] + t * P:KOFF[ck] + (t + 1) * P, :])
                        crec = work.tile([P, 1], F32, name="crec", tag="crec", bufs=2)
                        h = nc.vector.reciprocal(crec, rsb[:, D:D + 1])
                        add_dep_helper(h.ins, o1_gate[fin_gate[ck]].ins, False,
                                       "fused2 finalize after late panel")
                        f2s = work.tile([P, D], F32, name="f2s", tag="f2s", bufs=2)
                        nc.vector.tensor_scalar(
                            out=f2s, in0=rsb[:, 0:D], scalar1=crec,
                            scalar2=None, op0=ALU.mult)
                        nc.vector.tensor_add(f2s, f2s, km)
                        r0 = KOFF[ck] + t * P
                        nc.sync.dma_start(out=f2_o[r0:r0 + P, :], in_=f2s)

    nc.compile()
    return nc


def _get_nc():
    if "nc" not in _nc_cache:
        _nc_cache["nc"] = _build_nc()
    return _nc_cache["nc"]


def _make_in_maps(h1, h2, Wq, bq, Wk, bk):
    h1 = np.ascontiguousarray(h1, np.float32)
    h2 = np.ascontiguousarray(h2, np.float32)
    s = np.float32(1.0 / np.sqrt(D))
    wqt = np.ascontiguousarray(np.asarray(Wq, np.float32).T)
    in_common = {
        "wqt_s": wqt * s,
        "wqt": wqt,
        "wkt": np.ascontiguousarray(np.asarray(Wk, np.float32).T),
        "bq_s": (np.asarray(bq, np.float32) * s).reshape(1, D),
        "bq": np.asarray(bq, np.float32).reshape(1, D),
        "bk": np.asarray(bk, np.float32).reshape(1, D),
    }
    in_common["h2h"] = h2[0:PANEL]
    return [
        {"h1": h1[c * NLOC:(c + 1) * NLOC], "h2": h2[c * NLOC:(c + 1) * NLOC],
         **in_common}
        for c in range(NCORES)
    ]


def _assemble(res):
    attn = np.concatenate([r["attn"] for r in res], axis=0)
    fused1 = np.concatenate([r["f1"] for r in res], axis=0)
    fused2 = np.empty((M, D), np.float32)
    for c in range(NCORES):
        f2c = res[c]["f2"]
        for ck in range(NCHUNKS):
            g0 = CHUNK_BASE[ck] + c * BAND[ck]
            fused2[g0:g0 + BAND[ck]] = f2c[KOFF[ck]:KOFF[ck] + BAND[ck]]
    return fused1, fused2, attn


def kernel(h1, h2, Wq, bq, Wk, bk):
    from concourse.bass_utils import run_bass_kernel_spmd

    in_maps = _make_in_maps(h1, h2, Wq, bq, Wk, bk)
    nc = _get_nc()
    res = run_bass_kernel_spmd(nc, in_maps, core_ids=list(range(NCORES))).results
    return _assemble(res)
